# revision 68
# baseline (speedup 1.0000x reference)
"""Cross-attention kernel for Trainium2 (8 NeuronCores, batch-parallel).

Reference computation (per batch element b):
    q = x @ Wq + bq            # [T, E]
    k = y @ Wk + bk            # [S, E]
    v = y @ Wv + bv            # [S, E]
    per head h (D=80): scores = q_h @ k_h.T / sqrt(D); A = softmax(scores)
    attn = concat_h(A @ v_h)   # [T, E]
    out = attn @ Wo + bo       # [T, E]

Sharding: batch (8) across the 8 cores, one batch element per core.

On-chip layout is feature-major (x and the output are transposed on the
host so every DMA is a contiguous row load/store and no on-chip
transposes are needed). All matmuls and DMAs run in bf16 (same PE rate
as float32r per the TRN2 cost model AND measured on HW, but half the
HBM traffic / SBUF pressure, plus 2x DVE mode); PSUM accumulation stays
fp32, so the only loss is bf16 rounding of operands (~5e-3 rel on the
output vs the 2e-2 gate). fp8 DoubleRow was measured on HW at ~1101ns
per Qproj-shaped chain vs bf16's 942ns — slower despite the cost
model's 0.5 cyc/col claim — so everything stays bf16.

Schedule: software pipeline two chunks deep. PE order per iteration i:
    scores_i -> Qproj_{i+2} -> AV_i -> Oproj_i
so chunk i's softmax normalization chain — s8 ones-matmul -> DVE
approx-reciprocal -> bf16 round -> DRAM bounce -> one partition-stride-0
broadcast-load -> per-head DVE multiplies — runs on the side engines
underneath Qproj_{i+2}'s ~5us of PE work. When Qproj runs out (the last
two chunks), the PREVIOUS chunk's O-projection is deferred into that
slot instead, and those chunks normalize POST-AV: 5 PE matmuls build a
per-feature-tile reciprocal map zrec_j = selj^T rec8b and the softmax
division folds into the PSUM->SBUF multiply, replacing 8 pre-AV
multiplies + 5 Act copies + the broadcast DMA entirely. The fill-phase
Qproj for chunks 0/1 and the K-projection run c-major (borrowing the
idle pav/pop PSUM banks, wk/wv loaded in two halves) so PE consumes
weight tiles in wire-arrival order instead of waiting for whole
tensors.

DMA discipline: HWDGE descriptor generation costs ~625ns per dma_start
and all wire transfers share one ~360GB/s stream, so everything is
batched (one DMA per x chunk / per output chunk / per broadcast) and
the per-chunk output store is emitted only after the next chunk's
broadcast-load so it queues behind it on the wire. Weight loads ride
the Activation-engine HWDGE ring; latency-critical x / softmax traffic
rides the SP ring.
"""

import numpy as np

import concourse.bass as bass
import concourse.bass_isa as bass_isa
import concourse.mybir as mybir
import concourse.tile as tile
from concourse import bacc
from concourse.bass_utils import run_bass_kernel_spmd

F32 = mybir.dt.float32
BF16 = mybir.dt.bfloat16
AF = mybir.ActivationFunctionType

B, T, E, CR, H, D, S = 8, 4096, 640, 768, 8, 80, 77
CHUNKS = [(512 * i, 512) for i in range(8)]
TC = 512                 # max token chunk (psum sizing)
NT = len(CHUNKS)
EJ = E // 128            # 5 e-tiles
CJ = CR // 128           # 6 cross-dim chunks
SCALE = float(1.0 / np.sqrt(D))
NSEL = 1                 # trailing chunks that broadcast via PE sel-matmul


def _frags():
    fr = []
    for h in range(H):
        e0, e1 = D * h, D * h + D
        for j in range(e0 // 128, (e1 - 1) // 128 + 1):
            p0, p1 = max(0, e0 - 128 * j), min(128, e1 - 128 * j)
            fr.append((h, j, p0, p1))
    return fr


FRAGS = _frags()
NF = len(FRAGS)


def _emit(nc, tc, dr):
    import contextlib

    ctx = contextlib.ExitStack()
    with ctx:
        cpool = ctx.enter_context(tc.tile_pool(name="const", bufs=1))
        pq = ctx.enter_context(tc.tile_pool(name="pq", bufs=2, space="PSUM"))
        psc = ctx.enter_context(tc.tile_pool(name="psc", bufs=2, space="PSUM"))
        ps8 = ctx.enter_context(tc.tile_pool(name="ps8", bufs=1, space="PSUM"))
        pav = ctx.enter_context(tc.tile_pool(name="pav", bufs=2, space="PSUM"))
        pop = ctx.enter_context(tc.tile_pool(name="pop", bufs=1, space="PSUM"))
        xpool = ctx.enter_context(tc.tile_pool(name="xpool", bufs=2))
        qpool = ctx.enter_context(tc.tile_pool(name="qpool", bufs=3))
        apool = ctx.enter_context(tc.tile_pool(name="apool", bufs=2))
        bcpool = ctx.enter_context(tc.tile_pool(name="bcpool", bufs=2))
        rpool = ctx.enter_context(tc.tile_pool(name="rpool", bufs=3))
        atpool = ctx.enter_context(tc.tile_pool(name="atpool", bufs=2))
        opool = ctx.enter_context(tc.tile_pool(name="opool", bufs=2))

        def load(name, shape, src, dt=BF16):
            t = cpool.tile(shape, dt, tag=name, name=name)
            nc.scalar.dma_start(t[:], src)
            return t

        def loadw(name, src2, nblk, cols, dt=BF16):
            t = cpool.tile([128, nblk, cols], dt, tag=name, name=name)
            nc.scalar.dma_start(
                t[:], src2.rearrange("(b p) c -> p b c", p=128))
            return t

        def load_x(it, split_first=False):
            t0, tw = CHUNKS[it]
            xt = xpool.tile([128, EJ, TC], BF16, tag="xp", name=f"xp{it}")
            if split_first:
                # c=0 / c=1 / rest on separate DMAs so the c-major fill
                # Qproj never waits for more wire than the tile it needs
                nc.sync.dma_start(xt[:, 0, 0:tw], dr["xt"][0:128, t0:t0 + tw])
                nc.sync.dma_start(xt[:, 1, 0:tw],
                                  dr["xt"][128:256, t0:t0 + tw])
                nc.sync.dma_start(
                    xt[:, 2:EJ, 0:tw],
                    dr["xt"][256:E, t0:t0 + tw].rearrange(
                        "(b p) c -> p b c", p=128))
            else:
                nc.sync.dma_start(
                    xt[:, :, 0:tw],
                    dr["xt"][:, t0:t0 + tw].rearrange("(b p) c -> p b c",
                                                      p=128))
            return xt

        # ---- load constants/weights ----
        # load order matters: it sets HWDGE/DMA order, which gates when PE
        # can start. wq (split per c-tile) + chunk-0/1 x first, wo last.
        wq_sb = []
        for c in range(EJ):
            if c == 0:
                t = cpool.tile([128, E], BF16, tag="wq0", name="wq0")
                nc.sync.dma_start(t[:], dr["wq"][0:128, :])
                wq_sb.append(t)
                x_cur = load_x(0, split_first=True)
            else:
                wq_sb.append(load(f"wq{c}", [128, E],
                              dr["wq"][128 * c:128 * (c + 1), :]))
        fc = load("fc", [128, 15 + NF], dr["fc"][:], F32)
        bqt, bkt, bot = fc[:, 0:EJ], fc[:, EJ:2 * EJ], fc[:, 2 * EJ:3 * EJ]
        kmask = fc[:, 3 * EJ:3 * EJ + NF]
        ytp_t = loadw("yt", dr["yt"][:], CJ, S)
        x_nxt = load_x(1)
        wk_t = cpool.tile([128, CJ, E], BF16, tag="wk", name="wk")
        nc.scalar.dma_start(
            wk_t[:, 0:3, :],
            dr["wk"][0:384, :].rearrange("(b p) c -> p b c", p=128))
        nc.scalar.dma_start(
            wk_t[:, 3:CJ, :],
            dr["wk"][384:CR, :].rearrange("(b p) c -> p b c", p=128))
        wv_t = cpool.tile([128, CJ, E], BF16, tag="wv", name="wv")
        nc.scalar.dma_start(
            wv_t[:, 0:3, :],
            dr["wv"][0:384, :].rearrange("(b p) c -> p b c", p=128))
        nc.scalar.dma_start(
            wv_t[:, 3:CJ, :],
            dr["wv"][384:CR, :].rearrange("(b p) c -> p b c", p=128))
        fb1 = load("fb1", [1, E + S], dr["fb1"][:])
        bvr, ones77 = fb1[:, 0:E], fb1[:, E:E + S]
        e77 = load("e77", [S, H * H + H * S + 128 * NF + E], dr["e77"][:])
        ecols = e77[:, 0:H * H]
        sel8_sb = e77[0:H, H * H:H * H + H * S]
        vmask = e77[:, H * H + H * S:H * H + H * S + 128 * NF]
        selj_sb = e77[0:H, H * H + H * S + 128 * NF:]
        wo_t = loadw("wo", dr["wo"][:], EJ, E)
        wk_sb = [wk_t[:, c, :] for c in range(CJ)]
        wv_sb = [wv_t[:, c, :] for c in range(CJ)]
        wo_sb = [wo_t[:, c, :] for c in range(EJ)]
        ytp = [ytp_t[:, c, :] for c in range(CJ)]

        def emit_qproj(xp, tw):
            qs = []
            for j in range(EJ):
                qp = pq.tile([128, TC], F32, tag="qp", name=f"qp{j}")
                for c in range(EJ):
                    nc.tensor.matmul(qp[0:128, 0:tw],
                                     wq_sb[c][:, 128 * j:128 * (j + 1)],
                                     xp[:, c, 0:tw],
                                     start=(c == 0), stop=(c == EJ - 1))
                q = qpool.tile([128, TC], BF16, tag=f"q{j}", name=f"q{j}")
                nc.vector.tensor_scalar_add(q[0:128, 0:tw], qp[0:128, 0:tw],
                                            bqt[:, j:j + 1])
                qs.append(q)
            return qs

        def emit_qproj_cmajor(xp, tw):
            # fill-phase variant: c-major loop order so PE consumes wq
            # c-tiles in wire-arrival order (j-major needs ALL five tiles
            # before the first PSUM group can finish). Borrows the idle
            # pav/pop banks so all 5 output groups are live at once.
            qps = [pq.tile([128, TC], F32, tag="qp", name=f"qpa{j}")
                   for j in range(2)]
            qps += [pav.tile([128, TC], F32, tag="av", name=f"qpb{j}")
                    for j in range(2)]
            qps.append(pop.tile([128, TC], F32, tag="op", name="qpc"))
            for c in range(EJ):
                for j in range(EJ):
                    nc.tensor.matmul(qps[j][0:128, 0:tw],
                                     wq_sb[c][:, 128 * j:128 * (j + 1)],
                                     xp[:, c, 0:tw],
                                     start=(c == 0), stop=(c == EJ - 1))
            qs = []
            for j in range(EJ):
                q = qpool.tile([128, TC], BF16, tag=f"q{j}", name=f"q{j}")
                nc.vector.tensor_scalar_add(q[0:128, 0:tw],
                                            qps[j][0:128, 0:tw],
                                            bqt[:, j:j + 1])
                qs.append(q)
            return qs

        # chunk-0/1 Q-projections first: PE is in-order, so these must
        # precede the K/V setup in program order to start as soon as
        # wq+x arrive, and they cover the K/V weight-load wire time.
        qs_cur = emit_qproj_cmajor(x_cur, CHUNKS[0][1])
        qs_nxt = emit_qproj_cmajor(x_nxt, CHUNKS[1][1])

        # ---- K projection -> zero-padded per-fragment staging tiles ----
        # kstage[fi] = (k'_tile + bk) * mask_fi  (mask zeroes rows outside
        # the head fragment; done full-partition because engine ops need
        # 32-aligned start partitions)
        kstage = [cpool.tile([128, S], BF16, tag=f"ks{fi}", name=f"ks{fi}")
                  for fi in range(NF)]
        # c-major: PE consumes each wk c-half as it arrives off the wire
        # instead of waiting for the whole tensor (kp_j banks are all idle
        # during the fill phase)
        kps = []
        for j in range(EJ):
            kpool, ktag = [(pq, "qp"), (pq, "qp"), (pop, "op"),
                           (pav, "av"), (pav, "av")][j]
            kps.append(kpool.tile([128, S], F32, tag=ktag, name=f"kp{j}"))
        for c in range(CJ):
            for j in range(EJ):
                nc.tensor.matmul(kps[j][:],
                                 wk_sb[c][:, 128 * j:128 * (j + 1)],
                                 ytp[c], start=(c == 0), stop=(c == CJ - 1))
        for j in range(EJ):
            for fi, (h, jj, p0, p1) in enumerate(FRAGS):
                if jj != j:
                    continue
                nc.vector.tensor_scalar(kstage[fi][:], kps[j][:],
                                        bkt[:, j:j + 1], kmask[:, fi:fi + 1],
                                        mybir.AluOpType.add,
                                        mybir.AluOpType.mult)

        # ---- V projection (+bias via K=1 ones matmul) -> vb fragments ----
        vb = [cpool.tile([S, 128], BF16, tag=f"vb{fi}", name=f"vb{fi}")
              for fi in range(NF)]
        if True:  # (kept indented to match the emitted-block structure)
            for (n0, n1) in ((0, 512), (512, E)):
                vp = psc.tile([S, n1 - n0], F32, tag="sc")
                for c in range(CJ):
                    nc.tensor.matmul(vp[:], ytp[c], wv_sb[c][:, n0:n1],
                                     start=(c == 0), stop=False)
                nc.tensor.matmul(vp[:], ones77[:], bvr[:, n0:n1],
                                 start=False, stop=True)
                for fi, (h, j, p0, p1) in enumerate(FRAGS):
                    c0 = 128 * j
                    if not (n0 <= c0 and c0 + 128 <= n1):
                        continue
                    nc.vector.tensor_mul(vb[fi][:],
                                         vp[:, c0 - n0:c0 - n0 + 128],
                                         vmask[:, 128 * fi:128 * (fi + 1)])

        # ---- main loop over token chunks, software-pipelined ----
        def emit_scores(qs, tw):
            s8 = ps8.tile([H, TC], F32, tag="s8")
            aps = []
            for h in range(H):
                frs = [(fi, f) for fi, f in enumerate(FRAGS) if f[0] == h]
                sc = psc.tile([S, TC], F32, tag="sc")
                for i, (fi, (hh, j, p0, p1)) in enumerate(frs):
                    nc.tensor.matmul(sc[0:S, 0:tw], kstage[fi][:],
                                     qs[j][0:128, 0:tw],
                                     start=(i == 0), stop=(i == len(frs) - 1))
                a = apool.tile([S, TC], BF16, tag=f"a{h}")
                nc.scalar.activation(a[0:S, 0:tw], sc[0:S, 0:tw], AF.Exp,
                                     scale=SCALE)
                nc.tensor.matmul(s8[0:H, 0:tw], ecols[:, H * h:H * h + H],
                                 a[0:S, 0:tw],
                                 start=(h == 0), stop=(h == H - 1),
                                 skip_group_check=True)
                aps.append(a)
            return s8, aps

        def emit_recip(s8, tw):
            # approx 1/Z (fp32) + bf16 round + ONE flatten DMA of all 8
            # head rows into [1, H*tw] (partition_broadcast reads p0 only)
            rec8 = rpool.tile([H, TC], F32, tag="rec8")
            nc.vector.reciprocal_approx_fast(rec8[0:H, 0:tw], s8[0:H, 0:tw])
            rec8b = rpool.tile([H, TC], BF16, tag="rec8b")
            with nc.allow_low_precision(reason="softmax recip in bf16"):
                nc.vector.tensor_copy(rec8b[0:H, 0:tw], rec8[0:H, 0:tw])
            return rec8b

        def emit_bcast_dma(rec8b, it, tw):
            # bounce the [H, tw] reciprocal rows through DRAM, reloading
            # with a 0-stride partition dim: one store + one broadcast-load
            # replaces 8 serial Pool partition_broadcasts (Tile tracks the
            # DRAM store->load hazard through the DMA semaphores)
            zsl = dr["zs"][0:H, TC * it:TC * it + tw]
            nc.sync.dma_start(zsl, rec8b[0:H, 0:tw])
            bca = bcpool.tile([S, H, TC], BF16, tag="bca")
            nc.sync.dma_start(bca[:, :, 0:tw], zsl.partition_broadcast(S))
            return bca

        def emit_mult(aps, bca, tw):
            for h in range(H):
                nc.vector.tensor_mul(aps[h][0:S, 0:tw], aps[h][0:S, 0:tw],
                                     bca[:, h, 0:tw])

        def emit_mult_sel(aps, rec8b, tw):
            # last chunks: broadcast via PE sel-matmul (no Qproj left on PE
            # to hide the Pool-broadcast latency behind)
            for h in range(H):
                bc = psc.tile([S, TC], F32, tag="sc")
                nc.tensor.matmul(bc[0:S, 0:tw], sel8_sb[:, S * h:S * (h + 1)],
                                 rec8b[0:H, 0:tw])
                nc.vector.tensor_mul(aps[h][0:S, 0:tw], aps[h][0:S, 0:tw],
                                     bc[0:S, 0:tw])

        def emit_zrec(rec8b, tw):
            # tail-chunk path: per-feature-tile reciprocal maps
            # zrec_j = selj^T rec8b (5 PE matmuls), staged to SBUF bf16 via
            # Act copies (HW allows only ONE PSUM operand per DVE op, so the
            # later av*zrec multiply needs zrec in SBUF). Emitted before the
            # filler O-projection, so the copies are latency-hidden.
            zrs = []
            for j in range(EJ):
                zp = psc.tile([128, TC], F32, tag="sc", name=f"zrp{j}")
                nc.tensor.matmul(zp[0:128, 0:tw],
                                 selj_sb[:, 128 * j:128 * (j + 1)],
                                 rec8b[0:H, 0:tw])
                z = bcpool.tile([128, TC], BF16, tag=f"zr{j}")
                nc.scalar.activation(z[0:128, 0:tw], zp[0:128, 0:tw],
                                     AF.Copy)
                zrs.append(z)
            return zrs

        def emit_av(aps, tw, zrs=None):
            # zrs given: aps are UNnormalized; the softmax division folds
            # into the PSUM->SBUF multiply (replacing the Act copy)
            attn = []
            for j in range(EJ):
                av = pav.tile([128, TC], F32, tag="av")
                frs = [(fi, f) for fi, f in enumerate(FRAGS) if f[1] == j]
                for i, (fi, (h, jj, p0, p1)) in enumerate(frs):
                    nc.tensor.matmul(av[0:128, 0:tw], vb[fi][:],
                                     aps[h][0:S, 0:tw],
                                     start=(i == 0), stop=(i == len(frs) - 1))
                at = atpool.tile([128, TC], BF16, tag=f"at{j}")
                if zrs is not None:
                    with nc.allow_low_precision(reason="attn tile in bf16"):
                        nc.vector.tensor_mul(at[0:128, 0:tw],
                                             av[0:128, 0:tw],
                                             zrs[j][0:128, 0:tw])
                else:
                    nc.scalar.activation(at[0:128, 0:tw], av[0:128, 0:tw],
                                         AF.Copy)
                attn.append(at)
            return attn

        def emit_oproj(attn, it, t0, tw):
            # O projection + bias; batched out DMA except on the last chunk
            # (per-tile DMAs there so the store drain starts earlier).
            last = it == NT - 1
            ob5 = None if last else opool.tile([128, EJ, TC], BF16, tag="ob")
            for p in range(EJ):
                if it >= NT - 3 and p % 2 == 0:
                    op = pq.tile([128, TC], F32, tag="qp", name=f"opq{p}")
                elif it < NT - 3 and p % 2 == 0:
                    op = pav.tile([128, TC], F32, tag="av", name=f"opv{p}")
                else:
                    op = pop.tile([128, TC], F32, tag="op")
                for j in range(EJ):
                    nc.tensor.matmul(op[0:128, 0:tw],
                                     wo_sb[j][:, 128 * p:128 * (p + 1)],
                                     attn[j][0:128, 0:tw],
                                     start=(j == 0), stop=(j == EJ - 1))
                if last:
                    ob = opool.tile([128, TC], BF16, tag=f"obl{p}")
                    nc.scalar.activation(ob[0:128, 0:tw], op[0:128, 0:tw],
                                         AF.Identity, bias=bot[:, p:p + 1])
                    nc.sync.dma_start(
                        dr["ot"][128 * p:128 * (p + 1), t0:t0 + tw],
                        ob[0:128, 0:tw])
                else:
                    nc.scalar.activation(ob5[:, p, 0:tw], op[0:128, 0:tw],
                                         AF.Identity, bias=bot[:, p:p + 1])
            # the store DMA is flushed by the caller AFTER the next chunk's
            # broadcast-load is issued, so it queues behind it on the wire
            return None if last else (ob5, t0, tw)

        def flush_out(pending):
            if pending is not None:
                ob5, t0, tw = pending
                nc.scalar.dma_start(
                    dr["ot"][:, t0:t0 + tw].rearrange("(b p) c -> p b c",
                                                      p=128),
                    ob5[:, :, 0:tw])

        # Uniform pipeline: every chunk's softmax-normalization latency is
        # hidden behind ~5us of independent PE work — Qproj_{i+2} while it
        # exists, then the previous chunk's deferred O-projection.
        deferred = None          # (attn, it, t0, tw) awaiting O-projection
        pending_out = None       # (ob5, t0, tw) awaiting its store DMA
        for it in range(NT):
            t0, tw = CHUNKS[it]
            if it + 2 < NT:
                x_fut = load_x(it + 2)
            s8, aps = emit_scores(qs_cur, tw)
            rec8b = emit_recip(s8, tw)
            use_post = it >= NT - 2
            bca = None if use_post else emit_bcast_dma(rec8b, it, tw)
            zrs = emit_zrec(rec8b, tw) if use_post else None
            flush_out(pending_out)
            pending_out = None
            if it + 2 < NT:
                qs_cur, qs_nxt = qs_nxt, emit_qproj(x_fut, CHUNKS[it + 2][1])
            else:
                qs_cur = qs_nxt
                if deferred is not None:
                    pending_out = emit_oproj(*deferred)
                    deferred = None
            if not use_post:
                emit_mult(aps, bca, tw)
            if it == NT - 1 and pending_out is not None:
                flush_out(pending_out)
                pending_out = None
            attn = emit_av(aps, tw, zrs=zrs)
            if it < NT - 3:
                pending_out = emit_oproj(attn, it, t0, tw)
            elif it == NT - 1:
                emit_oproj(attn, it, t0, tw)
            else:
                deferred = (attn, it, t0, tw)
        flush_out(pending_out)


def build_program(iters=1):
    nc = bacc.Bacc("TRN2", target_bir_lowering=False, debug=False, num_devices=B)
    dr = {}

    def din(name, shape, dt=BF16):
        dr[name] = nc.dram_tensor(name, shape, dt, kind="ExternalInput")
        return dr[name]

    din("xt", [E, T])
    din("yt", [CR, S])
    din("wq", [E, E])
    din("wk", [CR, E])
    din("wv", [CR, E])
    din("wo", [E, E])
    din("fc", [128, 15 + NF], F32)
    din("fb1", [1, E + S])
    din("e77", [S, H * H + H * S + 128 * NF + E])
    dr["zs"] = nc.dram_tensor("zs", [H, NT * TC], BF16, kind="Internal")
    dr["ot"] = nc.dram_tensor("ot", [E, T], BF16, kind="ExternalOutput")

    with tile.TileContext(nc) as tc:
        for _ in range(iters):
            _emit(nc, tc, {k: v[:] for k, v in dr.items()})
    nc.compile()
    return nc


def make_in_maps(x, y, Wq, bq, Wk, bk, Wv, bv, Wo, bo):
    bf = mybir.dt.np(BF16)
    f32 = np.float32
    fb = lambda a: np.ascontiguousarray(np.asarray(a, dtype=f32).astype(bf))
    ecols = np.zeros((S, H * H), f32)
    for h in range(H):
        ecols[:, H * h + h] = 1.0
    sel8 = np.zeros((H, S * H), f32)
    for h in range(H):
        sel8[h, S * h:S * (h + 1)] = 1.0
    # sel8 lives in the [S,*] fused constant tensor: only rows 0..H-1 used
    sel8_pad = np.zeros((S, H * S), f32)
    sel8_pad[0:H, :] = sel8
    kmask = np.zeros((128, NF), f32)
    vmask = np.zeros((S, 128 * NF), f32)
    for fi, (h, j, p0, p1) in enumerate(FRAGS):
        kmask[p0:p1, fi] = 1.0
        vmask[:, 128 * fi + p0:128 * fi + p1] = 1.0
    fc = np.concatenate([
        np.asarray(bq, f32).reshape(EJ, 128).T,
        np.asarray(bk, f32).reshape(EJ, 128).T,
        np.asarray(bo, f32).reshape(EJ, 128).T,
        kmask,
    ], axis=1)
    fb1 = np.concatenate([np.asarray(bv, f32).reshape(1, E),
                          np.ones((1, S), f32)], axis=1)
    selj = np.zeros((S, E), f32)
    for e in range(E):
        selj[e // D, e] = 1.0          # rows 0..H-1 used (head of feature e)
    e77 = np.concatenate([ecols, sel8_pad, vmask, selj], axis=1)
    shared = dict(
        wq=fb(Wq), wk=fb(Wk), wv=fb(Wv), wo=fb(Wo),
        fc=np.ascontiguousarray(fc),
        fb1=fb(fb1),
        e77=fb(e77),
    )
    x = np.asarray(x, f32)
    y = np.asarray(y, f32)
    in_maps = []
    for b in range(B):
        m = dict(shared)
        m["xt"] = fb(x[b].T)
        m["yt"] = fb(y[b].T)
        in_maps.append(m)
    return in_maps


def assemble_output(results):
    return np.stack(
        [np.asarray(results[b]["ot"], dtype=np.float32).T for b in range(B)],
        axis=0)


_PROG = None


def _prog():
    global _PROG
    if _PROG is None:
        _PROG = build_program()
    return _PROG


def kernel(x, y, Wq, bq, Wk, bk, Wv, bv, Wo, bo):
    nc = _prog()
    in_maps = make_in_maps(x, y, Wq, bq, Wk, bk, Wv, bv, Wo, bo)
    res = run_bass_kernel_spmd(nc, in_maps, core_ids=list(range(B)))
    return assemble_output(res.results)


# revision 71
# speedup vs baseline: 1.0063x; 1.0063x over previous
"""Cross-attention kernel for Trainium2 (8 NeuronCores, batch-parallel).

Reference computation (per batch element b):
    q = x @ Wq + bq            # [T, E]
    k = y @ Wk + bk            # [S, E]
    v = y @ Wv + bv            # [S, E]
    per head h (D=80): scores = q_h @ k_h.T / sqrt(D); A = softmax(scores)
    attn = concat_h(A @ v_h)   # [T, E]
    out = attn @ Wo + bo       # [T, E]

Sharding: batch (8) across the 8 cores, one batch element per core.

On-chip layout is feature-major (x and the output are transposed on the
host so every DMA is a contiguous row load/store and no on-chip
transposes are needed). All matmuls and DMAs run in bf16 (same PE rate
as float32r per the TRN2 cost model AND measured on HW, but half the
HBM traffic / SBUF pressure, plus 2x DVE mode); PSUM accumulation stays
fp32, so the only loss is bf16 rounding of operands (~5e-3 rel on the
output vs the 2e-2 gate). fp8 DoubleRow was measured on HW at ~1101ns
per Qproj-shaped chain vs bf16's 942ns — slower despite the cost
model's 0.5 cyc/col claim — so everything stays bf16.

Schedule: software pipeline two chunks deep. PE order per iteration i:
    scores_i -> Qproj_{i+2} -> AV_i -> Oproj_i
so chunk i's softmax normalization chain — s8 ones-matmul -> DVE
approx-reciprocal -> bf16 round -> DRAM bounce -> one partition-stride-0
broadcast-load -> per-head DVE multiplies — runs on the side engines
underneath Qproj_{i+2}'s ~5us of PE work. When Qproj runs out (the last
two chunks), the PREVIOUS chunk's O-projection is deferred into that
slot instead, and those chunks normalize POST-AV: 5 PE matmuls build a
per-feature-tile reciprocal map zrec_j = selj^T rec8b and the softmax
division folds into the PSUM->SBUF multiply, replacing 8 pre-AV
multiplies + 5 Act copies + the broadcast DMA entirely. The fill-phase
Qproj for chunks 0/1 and the K-projection run c-major (borrowing the
idle pav/pop PSUM banks, wk/wv loaded in two halves) so PE consumes
weight tiles in wire-arrival order instead of waiting for whole
tensors.

DMA discipline: HWDGE descriptor generation costs ~625ns per dma_start
and all wire transfers share one ~360GB/s stream, so everything is
batched (one DMA per x chunk / per output chunk / per broadcast) and
the per-chunk output store is emitted only after the next chunk's
broadcast-load so it queues behind it on the wire. Weight loads ride
the Activation-engine HWDGE ring; latency-critical x / softmax traffic
rides the SP ring.
"""

import numpy as np

import concourse.bass as bass
import concourse.bass_isa as bass_isa
import concourse.mybir as mybir
import concourse.tile as tile
from concourse import bacc
from concourse.bass_utils import run_bass_kernel_spmd

F32 = mybir.dt.float32
BF16 = mybir.dt.bfloat16
AF = mybir.ActivationFunctionType

B, T, E, CR, H, D, S = 8, 4096, 640, 768, 8, 80, 77
CHUNKS = [(512 * i, 512) for i in range(8)]
TC = 512                 # max token chunk (psum sizing)
NT = len(CHUNKS)
EJ = E // 128            # 5 e-tiles
CJ = CR // 128           # 6 cross-dim chunks
SCALE = float(1.0 / np.sqrt(D))
NSEL = 1                 # trailing chunks that broadcast via PE sel-matmul


def _frags():
    fr = []
    for h in range(H):
        e0, e1 = D * h, D * h + D
        for j in range(e0 // 128, (e1 - 1) // 128 + 1):
            p0, p1 = max(0, e0 - 128 * j), min(128, e1 - 128 * j)
            fr.append((h, j, p0, p1))
    return fr


FRAGS = _frags()
NF = len(FRAGS)


def _emit(nc, tc, dr):
    import contextlib

    ctx = contextlib.ExitStack()
    with ctx:
        cpool = ctx.enter_context(tc.tile_pool(name="const", bufs=1))
        pq = ctx.enter_context(tc.tile_pool(name="pq", bufs=2, space="PSUM"))
        psc = ctx.enter_context(tc.tile_pool(name="psc", bufs=2, space="PSUM"))
        ps8 = ctx.enter_context(tc.tile_pool(name="ps8", bufs=1, space="PSUM"))
        pav = ctx.enter_context(tc.tile_pool(name="pav", bufs=2, space="PSUM"))
        pop = ctx.enter_context(tc.tile_pool(name="pop", bufs=1, space="PSUM"))
        xpool = ctx.enter_context(tc.tile_pool(name="xpool", bufs=2))
        qpool = ctx.enter_context(tc.tile_pool(name="qpool", bufs=3))
        apool = ctx.enter_context(tc.tile_pool(name="apool", bufs=2))
        bcpool = ctx.enter_context(tc.tile_pool(name="bcpool", bufs=2))
        rpool = ctx.enter_context(tc.tile_pool(name="rpool", bufs=3))
        atpool = ctx.enter_context(tc.tile_pool(name="atpool", bufs=2))
        opool = ctx.enter_context(tc.tile_pool(name="opool", bufs=2))

        def load(name, shape, src, dt=BF16):
            t = cpool.tile(shape, dt, tag=name, name=name)
            nc.scalar.dma_start(t[:], src)
            return t

        def loadw(name, src2, nblk, cols, dt=BF16):
            t = cpool.tile([128, nblk, cols], dt, tag=name, name=name)
            nc.scalar.dma_start(
                t[:], src2.rearrange("(b p) c -> p b c", p=128))
            return t

        def load_x(it, split_first=False):
            t0, tw = CHUNKS[it]
            xt = xpool.tile([128, EJ, TC], BF16, tag="xp", name=f"xp{it}")
            if split_first:
                # c=0 / c=1 / rest on separate DMAs so the c-major fill
                # Qproj never waits for more wire than the tile it needs
                nc.sync.dma_start(xt[:, 0, 0:tw], dr["xt"][0:128, t0:t0 + tw])
                nc.sync.dma_start(xt[:, 1, 0:tw],
                                  dr["xt"][128:256, t0:t0 + tw])
                nc.sync.dma_start(
                    xt[:, 2:EJ, 0:tw],
                    dr["xt"][256:E, t0:t0 + tw].rearrange(
                        "(b p) c -> p b c", p=128))
            else:
                nc.sync.dma_start(
                    xt[:, :, 0:tw],
                    dr["xt"][:, t0:t0 + tw].rearrange("(b p) c -> p b c",
                                                      p=128))
            return xt

        # ---- load constants/weights ----
        # load order matters: it sets HWDGE/DMA order, which gates when PE
        # can start. wq (split per c-tile) + chunk-0/1 x first, wo last.
        wq_sb = []
        for c in range(EJ):
            if c == 0:
                t = cpool.tile([128, E], BF16, tag="wq0", name="wq0")
                nc.sync.dma_start(t[:], dr["wq"][0:128, :])
                wq_sb.append(t)
                x_cur = load_x(0, split_first=True)
            else:
                wq_sb.append(load(f"wq{c}", [128, E],
                              dr["wq"][128 * c:128 * (c + 1), :]))
        fc = load("fc", [128, 15 + NF], dr["fc"][:], F32)
        bqt, bkt, bot = fc[:, 0:EJ], fc[:, EJ:2 * EJ], fc[:, 2 * EJ:3 * EJ]
        kmask = fc[:, 3 * EJ:3 * EJ + NF]
        ytp_t = loadw("yt", dr["yt"][:], CJ, S)
        x_nxt = load_x(1)
        wk_t = cpool.tile([128, CJ, E], BF16, tag="wk", name="wk")
        nc.scalar.dma_start(
            wk_t[:, 0:3, :],
            dr["wk"][0:384, :].rearrange("(b p) c -> p b c", p=128))
        nc.scalar.dma_start(
            wk_t[:, 3:CJ, :],
            dr["wk"][384:CR, :].rearrange("(b p) c -> p b c", p=128))
        wv_t = cpool.tile([128, CJ, E], BF16, tag="wv", name="wv")
        nc.scalar.dma_start(
            wv_t[:, 0:3, :],
            dr["wv"][0:384, :].rearrange("(b p) c -> p b c", p=128))
        nc.scalar.dma_start(
            wv_t[:, 3:CJ, :],
            dr["wv"][384:CR, :].rearrange("(b p) c -> p b c", p=128))
        fb1 = load("fb1", [1, E + S], dr["fb1"][:])
        bvr, ones77 = fb1[:, 0:E], fb1[:, E:E + S]
        e77 = load("e77", [S, H * H + H * S + 128 * NF + E], dr["e77"][:])
        ecols = e77[:, 0:H * H]
        sel8_sb = e77[0:H, H * H:H * H + H * S]
        vmask = e77[:, H * H + H * S:H * H + H * S + 128 * NF]
        selj_sb = e77[0:H, H * H + H * S + 128 * NF:]
        wo_t = loadw("wo", dr["wo"][:], EJ, E)
        wk_sb = [wk_t[:, c, :] for c in range(CJ)]
        wv_sb = [wv_t[:, c, :] for c in range(CJ)]
        wo_sb = [wo_t[:, c, :] for c in range(EJ)]
        ytp = [ytp_t[:, c, :] for c in range(CJ)]

        def emit_qproj(xp, tw):
            qs = []
            for j in range(EJ):
                qp = pq.tile([128, TC], F32, tag="qp", name=f"qp{j}")
                for c in range(EJ):
                    nc.tensor.matmul(qp[0:128, 0:tw],
                                     wq_sb[c][:, 128 * j:128 * (j + 1)],
                                     xp[:, c, 0:tw],
                                     start=(c == 0), stop=(c == EJ - 1))
                q = qpool.tile([128, TC], BF16, tag=f"q{j}", name=f"q{j}")
                nc.vector.tensor_scalar_add(q[0:128, 0:tw], qp[0:128, 0:tw],
                                            bqt[:, j:j + 1])
                qs.append(q)
            return qs

        def emit_qproj_cmajor(xp, tw):
            # fill-phase variant: c-major loop order so PE consumes wq
            # c-tiles in wire-arrival order (j-major needs ALL five tiles
            # before the first PSUM group can finish). Borrows the idle
            # pav/pop banks so all 5 output groups are live at once.
            qps = [pq.tile([128, TC], F32, tag="qp", name=f"qpa{j}")
                   for j in range(2)]
            qps += [pav.tile([128, TC], F32, tag="av", name=f"qpb{j}")
                    for j in range(2)]
            qps.append(pop.tile([128, TC], F32, tag="op", name="qpc"))
            for c in range(EJ):
                for j in range(EJ):
                    nc.tensor.matmul(qps[j][0:128, 0:tw],
                                     wq_sb[c][:, 128 * j:128 * (j + 1)],
                                     xp[:, c, 0:tw],
                                     start=(c == 0), stop=(c == EJ - 1))
            qs = []
            for j in range(EJ):
                q = qpool.tile([128, TC], BF16, tag=f"q{j}", name=f"q{j}")
                nc.vector.tensor_scalar_add(q[0:128, 0:tw],
                                            qps[j][0:128, 0:tw],
                                            bqt[:, j:j + 1])
                qs.append(q)
            return qs

        # chunk-0/1 Q-projections first: PE is in-order, so these must
        # precede the K/V setup in program order to start as soon as
        # wq+x arrive, and they cover the K/V weight-load wire time.
        qs_cur = emit_qproj_cmajor(x_cur, CHUNKS[0][1])
        qs_nxt = emit_qproj_cmajor(x_nxt, CHUNKS[1][1])

        # ---- K projection -> zero-padded per-fragment staging tiles ----
        # kstage[fi] = (k'_tile + bk) * mask_fi  (mask zeroes rows outside
        # the head fragment; done full-partition because engine ops need
        # 32-aligned start partitions)
        kstage = [cpool.tile([128, S], BF16, tag=f"ks{fi}", name=f"ks{fi}")
                  for fi in range(NF)]
        # c-major: PE consumes each wk c-half as it arrives off the wire
        # instead of waiting for the whole tensor (kp_j banks are all idle
        # during the fill phase)
        kps = []
        for j in range(EJ):
            kpool, ktag = [(pq, "qp"), (pq, "qp"), (pop, "op"),
                           (pav, "av"), (pav, "av")][j]
            kps.append(kpool.tile([128, S], F32, tag=ktag, name=f"kp{j}"))
        for c in range(CJ):
            for j in range(EJ):
                nc.tensor.matmul(kps[j][:],
                                 wk_sb[c][:, 128 * j:128 * (j + 1)],
                                 ytp[c], start=(c == 0), stop=(c == CJ - 1))
        for j in range(EJ):
            for fi, (h, jj, p0, p1) in enumerate(FRAGS):
                if jj != j:
                    continue
                nc.vector.tensor_scalar(kstage[fi][:], kps[j][:],
                                        bkt[:, j:j + 1], kmask[:, fi:fi + 1],
                                        mybir.AluOpType.add,
                                        mybir.AluOpType.mult)

        # ---- V projection (+bias via K=1 ones matmul) -> vb fragments ----
        vb = [cpool.tile([S, 128], BF16, tag=f"vb{fi}", name=f"vb{fi}")
              for fi in range(NF)]
        if True:  # (kept indented to match the emitted-block structure)
            for (n0, n1) in ((0, 512), (512, E)):
                vp = psc.tile([S, n1 - n0], F32, tag="sc")
                for c in range(CJ):
                    nc.tensor.matmul(vp[:], ytp[c], wv_sb[c][:, n0:n1],
                                     start=(c == 0), stop=False)
                nc.tensor.matmul(vp[:], ones77[:], bvr[:, n0:n1],
                                 start=False, stop=True)
                for fi, (h, j, p0, p1) in enumerate(FRAGS):
                    c0 = 128 * j
                    if not (n0 <= c0 and c0 + 128 <= n1):
                        continue
                    nc.vector.tensor_mul(vb[fi][:],
                                         vp[:, c0 - n0:c0 - n0 + 128],
                                         vmask[:, 128 * fi:128 * (fi + 1)])

        # ---- main loop over token chunks, software-pipelined ----
        def emit_scores(qs, tw, first_chunk=False):
            s8 = ps8.tile([H, TC], F32, tag="s8")
            aps = []
            for h in range(H):
                frs = [(fi, f) for fi, f in enumerate(FRAGS) if f[0] == h]
                if first_chunk and h < 2:
                    # psc banks are still being drained by V-proj's mask
                    # multiplies; pq/pop are free after K-proj
                    spool, stag = [(pq, "qp"), (pop, "op")][h]
                    sc = spool.tile([S, TC], F32, tag=stag, name=f"sc{h}")
                else:
                    sc = psc.tile([S, TC], F32, tag="sc")
                for i, (fi, (hh, j, p0, p1)) in enumerate(frs):
                    nc.tensor.matmul(sc[0:S, 0:tw], kstage[fi][:],
                                     qs[j][0:128, 0:tw],
                                     start=(i == 0), stop=(i == len(frs) - 1))
                a = apool.tile([S, TC], BF16, tag=f"a{h}")
                nc.scalar.activation(a[0:S, 0:tw], sc[0:S, 0:tw], AF.Exp,
                                     scale=SCALE)
                nc.tensor.matmul(s8[0:H, 0:tw], ecols[:, H * h:H * h + H],
                                 a[0:S, 0:tw],
                                 start=(h == 0), stop=(h == H - 1),
                                 skip_group_check=True)
                aps.append(a)
            return s8, aps

        def emit_recip(s8, tw):
            # approx 1/Z (fp32) + bf16 round + ONE flatten DMA of all 8
            # head rows into [1, H*tw] (partition_broadcast reads p0 only)
            rec8 = rpool.tile([H, TC], F32, tag="rec8")
            nc.vector.reciprocal_approx_fast(rec8[0:H, 0:tw], s8[0:H, 0:tw])
            rec8b = rpool.tile([H, TC], BF16, tag="rec8b")
            with nc.allow_low_precision(reason="softmax recip in bf16"):
                nc.vector.tensor_copy(rec8b[0:H, 0:tw], rec8[0:H, 0:tw])
            return rec8b

        def emit_bcast_dma(rec8b, it, tw):
            # bounce the [H, tw] reciprocal rows through DRAM, reloading
            # with a 0-stride partition dim: one store + one broadcast-load
            # replaces 8 serial Pool partition_broadcasts (Tile tracks the
            # DRAM store->load hazard through the DMA semaphores)
            zsl = dr["zs"][0:H, TC * it:TC * it + tw]
            nc.sync.dma_start(zsl, rec8b[0:H, 0:tw])
            bca = bcpool.tile([S, H, TC], BF16, tag="bca")
            nc.sync.dma_start(bca[:, :, 0:tw], zsl.partition_broadcast(S))
            return bca

        def emit_mult(aps, bca, tw):
            for h in range(H):
                nc.vector.tensor_mul(aps[h][0:S, 0:tw], aps[h][0:S, 0:tw],
                                     bca[:, h, 0:tw])

        def emit_mult_sel(aps, rec8b, tw):
            # last chunks: broadcast via PE sel-matmul (no Qproj left on PE
            # to hide the Pool-broadcast latency behind)
            for h in range(H):
                bc = psc.tile([S, TC], F32, tag="sc")
                nc.tensor.matmul(bc[0:S, 0:tw], sel8_sb[:, S * h:S * (h + 1)],
                                 rec8b[0:H, 0:tw])
                nc.vector.tensor_mul(aps[h][0:S, 0:tw], aps[h][0:S, 0:tw],
                                     bc[0:S, 0:tw])

        def emit_zrec(rec8b, tw):
            # tail-chunk path: per-feature-tile reciprocal maps
            # zrec_j = selj^T rec8b (5 PE matmuls), staged to SBUF bf16 via
            # Act copies (HW allows only ONE PSUM operand per DVE op, so the
            # later av*zrec multiply needs zrec in SBUF). Emitted before the
            # filler O-projection, so the copies are latency-hidden.
            zrs = []
            for j in range(EJ):
                zp = psc.tile([128, TC], F32, tag="sc", name=f"zrp{j}")
                nc.tensor.matmul(zp[0:128, 0:tw],
                                 selj_sb[:, 128 * j:128 * (j + 1)],
                                 rec8b[0:H, 0:tw])
                z = bcpool.tile([128, TC], BF16, tag=f"zr{j}")
                nc.scalar.activation(z[0:128, 0:tw], zp[0:128, 0:tw],
                                     AF.Copy)
                zrs.append(z)
            return zrs

        def emit_av(aps, tw, zrs=None):
            # zrs given: aps are UNnormalized; the softmax division folds
            # into the PSUM->SBUF multiply (replacing the Act copy)
            attn = []
            for j in range(EJ):
                av = pav.tile([128, TC], F32, tag="av")
                frs = [(fi, f) for fi, f in enumerate(FRAGS) if f[1] == j]
                for i, (fi, (h, jj, p0, p1)) in enumerate(frs):
                    nc.tensor.matmul(av[0:128, 0:tw], vb[fi][:],
                                     aps[h][0:S, 0:tw],
                                     start=(i == 0), stop=(i == len(frs) - 1))
                at = atpool.tile([128, TC], BF16, tag=f"at{j}")
                if zrs is not None:
                    with nc.allow_low_precision(reason="attn tile in bf16"):
                        nc.vector.tensor_mul(at[0:128, 0:tw],
                                             av[0:128, 0:tw],
                                             zrs[j][0:128, 0:tw])
                else:
                    nc.scalar.activation(at[0:128, 0:tw], av[0:128, 0:tw],
                                         AF.Copy)
                attn.append(at)
            return attn

        def emit_oproj(attn, it, t0, tw):
            # O projection + bias; batched out DMA except on the last chunk
            # (per-tile DMAs there so the store drain starts earlier).
            last = it == NT - 1
            ob5 = None if last else opool.tile([128, EJ, TC], BF16, tag="ob")
            for p in range(EJ):
                if it >= NT - 3 and p % 2 == 0:
                    op = pq.tile([128, TC], F32, tag="qp", name=f"opq{p}")
                elif it < NT - 3 and p % 2 == 0:
                    op = pav.tile([128, TC], F32, tag="av", name=f"opv{p}")
                else:
                    op = pop.tile([128, TC], F32, tag="op")
                for j in range(EJ):
                    nc.tensor.matmul(op[0:128, 0:tw],
                                     wo_sb[j][:, 128 * p:128 * (p + 1)],
                                     attn[j][0:128, 0:tw],
                                     start=(j == 0), stop=(j == EJ - 1))
                if last:
                    ob = opool.tile([128, TC], BF16, tag=f"obl{p}")
                    nc.scalar.activation(ob[0:128, 0:tw], op[0:128, 0:tw],
                                         AF.Identity, bias=bot[:, p:p + 1])
                    nc.sync.dma_start(
                        dr["ot"][128 * p:128 * (p + 1), t0:t0 + tw],
                        ob[0:128, 0:tw])
                else:
                    nc.scalar.activation(ob5[:, p, 0:tw], op[0:128, 0:tw],
                                         AF.Identity, bias=bot[:, p:p + 1])
            # the store DMA is flushed by the caller AFTER the next chunk's
            # broadcast-load is issued, so it queues behind it on the wire
            return None if last else (ob5, t0, tw)

        def flush_out(pending):
            if pending is not None:
                ob5, t0, tw = pending
                nc.scalar.dma_start(
                    dr["ot"][:, t0:t0 + tw].rearrange("(b p) c -> p b c",
                                                      p=128),
                    ob5[:, :, 0:tw])

        # Uniform pipeline: every chunk's softmax-normalization latency is
        # hidden behind ~5us of independent PE work — Qproj_{i+2} while it
        # exists, then the previous chunk's deferred O-projection.
        deferred = None          # (attn, it, t0, tw) awaiting O-projection
        pending_out = None       # (ob5, t0, tw) awaiting its store DMA
        for it in range(NT):
            t0, tw = CHUNKS[it]
            if it + 2 < NT:
                x_fut = load_x(it + 2)
            s8, aps = emit_scores(qs_cur, tw, first_chunk=(it == 0))
            rec8b = emit_recip(s8, tw)
            use_post = it >= NT - 2
            bca = None if use_post else emit_bcast_dma(rec8b, it, tw)
            zrs = emit_zrec(rec8b, tw) if use_post else None
            flush_out(pending_out)
            pending_out = None
            if it + 2 < NT:
                qs_cur, qs_nxt = qs_nxt, emit_qproj(x_fut, CHUNKS[it + 2][1])
            else:
                qs_cur = qs_nxt
                if deferred is not None:
                    pending_out = emit_oproj(*deferred)
                    deferred = None
            if not use_post:
                emit_mult(aps, bca, tw)
            if it == NT - 1 and pending_out is not None:
                flush_out(pending_out)
                pending_out = None
            attn = emit_av(aps, tw, zrs=zrs)
            if it < NT - 3:
                pending_out = emit_oproj(attn, it, t0, tw)
            elif it == NT - 1:
                emit_oproj(attn, it, t0, tw)
            else:
                deferred = (attn, it, t0, tw)
        flush_out(pending_out)


def build_program(iters=1):
    nc = bacc.Bacc("TRN2", target_bir_lowering=False, debug=False, num_devices=B)
    dr = {}

    def din(name, shape, dt=BF16):
        dr[name] = nc.dram_tensor(name, shape, dt, kind="ExternalInput")
        return dr[name]

    din("xt", [E, T])
    din("yt", [CR, S])
    din("wq", [E, E])
    din("wk", [CR, E])
    din("wv", [CR, E])
    din("wo", [E, E])
    din("fc", [128, 15 + NF], F32)
    din("fb1", [1, E + S])
    din("e77", [S, H * H + H * S + 128 * NF + E])
    dr["zs"] = nc.dram_tensor("zs", [H, NT * TC], BF16, kind="Internal")
    dr["ot"] = nc.dram_tensor("ot", [E, T], BF16, kind="ExternalOutput")

    with tile.TileContext(nc) as tc:
        for _ in range(iters):
            _emit(nc, tc, {k: v[:] for k, v in dr.items()})
    nc.compile()
    return nc


def make_in_maps(x, y, Wq, bq, Wk, bk, Wv, bv, Wo, bo):
    bf = mybir.dt.np(BF16)
    f32 = np.float32
    fb = lambda a: np.ascontiguousarray(np.asarray(a, dtype=f32).astype(bf))
    ecols = np.zeros((S, H * H), f32)
    for h in range(H):
        ecols[:, H * h + h] = 1.0
    sel8 = np.zeros((H, S * H), f32)
    for h in range(H):
        sel8[h, S * h:S * (h + 1)] = 1.0
    # sel8 lives in the [S,*] fused constant tensor: only rows 0..H-1 used
    sel8_pad = np.zeros((S, H * S), f32)
    sel8_pad[0:H, :] = sel8
    kmask = np.zeros((128, NF), f32)
    vmask = np.zeros((S, 128 * NF), f32)
    for fi, (h, j, p0, p1) in enumerate(FRAGS):
        kmask[p0:p1, fi] = 1.0
        vmask[:, 128 * fi + p0:128 * fi + p1] = 1.0
    fc = np.concatenate([
        np.asarray(bq, f32).reshape(EJ, 128).T,
        np.asarray(bk, f32).reshape(EJ, 128).T,
        np.asarray(bo, f32).reshape(EJ, 128).T,
        kmask,
    ], axis=1)
    fb1 = np.concatenate([np.asarray(bv, f32).reshape(1, E),
                          np.ones((1, S), f32)], axis=1)
    selj = np.zeros((S, E), f32)
    for e in range(E):
        selj[e // D, e] = 1.0          # rows 0..H-1 used (head of feature e)
    e77 = np.concatenate([ecols, sel8_pad, vmask, selj], axis=1)
    shared = dict(
        wq=fb(Wq), wk=fb(Wk), wv=fb(Wv), wo=fb(Wo),
        fc=np.ascontiguousarray(fc),
        fb1=fb(fb1),
        e77=fb(e77),
    )
    x = np.asarray(x, f32)
    y = np.asarray(y, f32)
    in_maps = []
    for b in range(B):
        m = dict(shared)
        m["xt"] = fb(x[b].T)
        m["yt"] = fb(y[b].T)
        in_maps.append(m)
    return in_maps


def assemble_output(results):
    return np.stack(
        [np.asarray(results[b]["ot"], dtype=np.float32).T for b in range(B)],
        axis=0)


_PROG = None


def _prog():
    global _PROG
    if _PROG is None:
        _PROG = build_program()
    return _PROG


def kernel(x, y, Wq, bq, Wk, bk, Wv, bv, Wo, bo):
    nc = _prog()
    in_maps = make_in_maps(x, y, Wq, bq, Wk, bk, Wv, bv, Wo, bo)
    res = run_bass_kernel_spmd(nc, in_maps, core_ids=list(range(B)))
    return assemble_output(res.results)


# revision 76
# speedup vs baseline: 1.0116x; 1.0053x over previous
"""Cross-attention kernel for Trainium2 (8 NeuronCores, batch-parallel).

Reference computation (per batch element b):
    q = x @ Wq + bq            # [T, E]
    k = y @ Wk + bk            # [S, E]
    v = y @ Wv + bv            # [S, E]
    per head h (D=80): scores = q_h @ k_h.T / sqrt(D); A = softmax(scores)
    attn = concat_h(A @ v_h)   # [T, E]
    out = attn @ Wo + bo       # [T, E]

Sharding: batch (8) across the 8 cores, one batch element per core.

On-chip layout is feature-major (x and the output are transposed on the
host so every DMA is a contiguous row load/store and no on-chip
transposes are needed). All matmuls and DMAs run in bf16 (same PE rate
as float32r per the TRN2 cost model AND measured on HW, but half the
HBM traffic / SBUF pressure, plus 2x DVE mode); PSUM accumulation stays
fp32, so the only loss is bf16 rounding of operands (~5e-3 rel on the
output vs the 2e-2 gate). fp8 DoubleRow was measured on HW at ~1101ns
per Qproj-shaped chain vs bf16's 942ns — slower despite the cost
model's 0.5 cyc/col claim — so everything stays bf16.

Schedule: software pipeline two chunks deep. PE order per iteration i:
    scores_i -> Qproj_{i+2} -> AV_i -> Oproj_i
so chunk i's softmax normalization chain — s8 ones-matmul -> DVE
approx-reciprocal -> bf16 round -> DRAM bounce -> one partition-stride-0
broadcast-load -> per-head DVE multiplies — runs on the side engines
underneath Qproj_{i+2}'s ~5us of PE work. When Qproj runs out (the last
two chunks), the PREVIOUS chunk's O-projection is deferred into that
slot instead, and those chunks normalize POST-AV: 5 PE matmuls build a
per-feature-tile reciprocal map zrec_j = selj^T rec8b and the softmax
division folds into the PSUM->SBUF multiply, replacing 8 pre-AV
multiplies + 5 Act copies + the broadcast DMA entirely. The fill-phase
Qproj for chunks 0/1 and the K-projection run c-major (borrowing the
idle pav/pop PSUM banks, wk/wv loaded in two halves) so PE consumes
weight tiles in wire-arrival order instead of waiting for whole
tensors.

DMA discipline: HWDGE descriptor generation costs ~625ns per dma_start
and all wire transfers share one ~360GB/s stream, so everything is
batched (one DMA per x chunk / per output chunk / per broadcast) and
the per-chunk output store is emitted only after the next chunk's
broadcast-load so it queues behind it on the wire. Weight loads ride
the Activation-engine HWDGE ring; latency-critical x / softmax traffic
rides the SP ring.
"""

import numpy as np

import concourse.bass as bass
import concourse.bass_isa as bass_isa
import concourse.mybir as mybir
import concourse.tile as tile
from concourse import bacc
from concourse.bass_utils import run_bass_kernel_spmd

F32 = mybir.dt.float32
BF16 = mybir.dt.bfloat16
AF = mybir.ActivationFunctionType

B, T, E, CR, H, D, S = 8, 4096, 640, 768, 8, 80, 77
CHUNKS = [(512 * i, 512) for i in range(8)]
TC = 512                 # max token chunk (psum sizing)
NT = len(CHUNKS)
EJ = E // 128            # 5 e-tiles
CJ = CR // 128           # 6 cross-dim chunks
SCALE = float(1.0 / np.sqrt(D))
NSEL = 1                 # trailing chunks that broadcast via PE sel-matmul


def _frags():
    fr = []
    for h in range(H):
        e0, e1 = D * h, D * h + D
        for j in range(e0 // 128, (e1 - 1) // 128 + 1):
            p0, p1 = max(0, e0 - 128 * j), min(128, e1 - 128 * j)
            fr.append((h, j, p0, p1))
    return fr


FRAGS = _frags()
NF = len(FRAGS)


def _emit(nc, tc, dr):
    import contextlib

    ctx = contextlib.ExitStack()
    with ctx:
        cpool = ctx.enter_context(tc.tile_pool(name="const", bufs=1))
        pq = ctx.enter_context(tc.tile_pool(name="pq", bufs=2, space="PSUM"))
        psc = ctx.enter_context(tc.tile_pool(name="psc", bufs=2, space="PSUM"))
        ps8 = ctx.enter_context(tc.tile_pool(name="ps8", bufs=1, space="PSUM"))
        pav = ctx.enter_context(tc.tile_pool(name="pav", bufs=2, space="PSUM"))
        pop = ctx.enter_context(tc.tile_pool(name="pop", bufs=1, space="PSUM"))
        xpool = ctx.enter_context(tc.tile_pool(name="xpool", bufs=2))
        qpool = ctx.enter_context(tc.tile_pool(name="qpool", bufs=3))
        apool = ctx.enter_context(tc.tile_pool(name="apool", bufs=2))
        bcpool = ctx.enter_context(tc.tile_pool(name="bcpool", bufs=2))
        rpool = ctx.enter_context(tc.tile_pool(name="rpool", bufs=3))
        atpool = ctx.enter_context(tc.tile_pool(name="atpool", bufs=2))
        opool = ctx.enter_context(tc.tile_pool(name="opool", bufs=2))

        def load(name, shape, src, dt=BF16):
            t = cpool.tile(shape, dt, tag=name, name=name)
            nc.scalar.dma_start(t[:], src)
            return t

        def loadw(name, src2, nblk, cols, dt=BF16):
            t = cpool.tile([128, nblk, cols], dt, tag=name, name=name)
            nc.scalar.dma_start(
                t[:], src2.rearrange("(b p) c -> p b c", p=128))
            return t

        def load_x(it, split_first=False):
            t0, tw = CHUNKS[it]
            xt = xpool.tile([128, EJ, TC], BF16, tag="xp", name=f"xp{it}")
            if split_first:
                # c=0 / c=1 / rest on separate DMAs so the c-major fill
                # Qproj never waits for more wire than the tile it needs
                nc.sync.dma_start(xt[:, 0, 0:tw], dr["xt"][0:128, t0:t0 + tw])
                nc.sync.dma_start(xt[:, 1, 0:tw],
                                  dr["xt"][128:256, t0:t0 + tw])
                nc.sync.dma_start(
                    xt[:, 2:EJ, 0:tw],
                    dr["xt"][256:E, t0:t0 + tw].rearrange(
                        "(b p) c -> p b c", p=128))
            else:
                nc.sync.dma_start(
                    xt[:, :, 0:tw],
                    dr["xt"][:, t0:t0 + tw].rearrange("(b p) c -> p b c",
                                                      p=128))
            return xt

        # ---- load constants/weights ----
        # load order matters: it sets HWDGE/DMA order, which gates when PE
        # can start. wq (split per c-tile) + chunk-0/1 x first, wo last.
        wq_sb = []
        for c in range(EJ):
            if c == 0:
                t = cpool.tile([128, E], BF16, tag="wq0", name="wq0")
                nc.sync.dma_start(t[:], dr["wq"][0:128, :])
                wq_sb.append(t)
                x_cur = load_x(0, split_first=True)
            else:
                wq_sb.append(load(f"wq{c}", [128, E],
                              dr["wq"][128 * c:128 * (c + 1), :]))
        fc = load("fc", [128, 15 + NF], dr["fc"][:], F32)
        bqt, bkt, bot = fc[:, 0:EJ], fc[:, EJ:2 * EJ], fc[:, 2 * EJ:3 * EJ]
        kmask = fc[:, 3 * EJ:3 * EJ + NF]
        ytp_t = loadw("yt", dr["yt"][:], CJ, S)
        x_nxt = load_x(1)
        wk_t = cpool.tile([128, CJ, E], BF16, tag="wk", name="wk")
        nc.scalar.dma_start(
            wk_t[:, 0:3, :],
            dr["wk"][0:384, :].rearrange("(b p) c -> p b c", p=128))
        nc.scalar.dma_start(
            wk_t[:, 3:CJ, :],
            dr["wk"][384:CR, :].rearrange("(b p) c -> p b c", p=128))
        wv_t = cpool.tile([128, CJ, E], BF16, tag="wv", name="wv")
        nc.scalar.dma_start(
            wv_t[:, 0:3, :],
            dr["wv"][0:384, :].rearrange("(b p) c -> p b c", p=128))
        nc.scalar.dma_start(
            wv_t[:, 3:CJ, :],
            dr["wv"][384:CR, :].rearrange("(b p) c -> p b c", p=128))
        fb1 = load("fb1", [1, E + S], dr["fb1"][:])
        bvr, ones77 = fb1[:, 0:E], fb1[:, E:E + S]
        e77 = load("e77", [S, H * H + H * S + 128 * NF + E], dr["e77"][:])
        ecols = e77[:, 0:H * H]
        sel8_sb = e77[0:H, H * H:H * H + H * S]
        vmask = e77[:, H * H + H * S:H * H + H * S + 128 * NF]
        selj_sb = e77[0:H, H * H + H * S + 128 * NF:]
        wo_t = loadw("wo", dr["wo"][:], EJ, E)
        wk_sb = [wk_t[:, c, :] for c in range(CJ)]
        wv_sb = [wv_t[:, c, :] for c in range(CJ)]
        wo_sb = [wo_t[:, c, :] for c in range(EJ)]
        ytp = [ytp_t[:, c, :] for c in range(CJ)]

        def emit_qproj(xp, tw):
            qs = []
            for j in range(EJ):
                qp = pq.tile([128, TC], F32, tag="qp", name=f"qp{j}")
                for c in range(EJ):
                    nc.tensor.matmul(qp[0:128, 0:tw],
                                     wq_sb[c][:, 128 * j:128 * (j + 1)],
                                     xp[:, c, 0:tw],
                                     start=(c == 0), stop=(c == EJ - 1))
                q = qpool.tile([128, TC], BF16, tag=f"q{j}", name=f"q{j}")
                nc.vector.tensor_scalar_add(q[0:128, 0:tw], qp[0:128, 0:tw],
                                            bqt[:, j:j + 1])
                qs.append(q)
            return qs

        def emit_qproj_cmajor(xp, tw):
            # fill-phase variant: c-major loop order so PE consumes wq
            # c-tiles in wire-arrival order (j-major needs ALL five tiles
            # before the first PSUM group can finish). Borrows the idle
            # pav/pop banks so all 5 output groups are live at once.
            qps = [pq.tile([128, TC], F32, tag="qp", name=f"qpa{j}")
                   for j in range(2)]
            qps += [pav.tile([128, TC], F32, tag="av", name=f"qpb{j}")
                    for j in range(2)]
            qps.append(pop.tile([128, TC], F32, tag="op", name="qpc"))
            for c in range(EJ):
                for j in range(EJ):
                    nc.tensor.matmul(qps[j][0:128, 0:tw],
                                     wq_sb[c][:, 128 * j:128 * (j + 1)],
                                     xp[:, c, 0:tw],
                                     start=(c == 0), stop=(c == EJ - 1))
            qs = []
            for j in range(EJ):
                q = qpool.tile([128, TC], BF16, tag=f"q{j}", name=f"q{j}")
                nc.vector.tensor_scalar_add(q[0:128, 0:tw],
                                            qps[j][0:128, 0:tw],
                                            bqt[:, j:j + 1])
                qs.append(q)
            return qs

        # chunk-0/1 Q-projections first: PE is in-order, so these must
        # precede the K/V setup in program order to start as soon as
        # wq+x arrive, and they cover the K/V weight-load wire time.
        qs_cur = emit_qproj_cmajor(x_cur, CHUNKS[0][1])
        qs_nxt = emit_qproj_cmajor(x_nxt, CHUNKS[1][1])

        # ---- K projection -> zero-padded per-fragment staging tiles ----
        # kstage[fi] = (k'_tile + bk) * mask_fi  (mask zeroes rows outside
        # the head fragment; done full-partition because engine ops need
        # 32-aligned start partitions)
        kstage = [cpool.tile([128, S], BF16, tag=f"ks{fi}", name=f"ks{fi}")
                  for fi in range(NF)]
        # c-major: PE consumes each wk c-half as it arrives off the wire
        # instead of waiting for the whole tensor (kp_j banks are all idle
        # during the fill phase)
        kps = []
        for j in range(EJ):
            kpool, ktag = [(pq, "qp"), (pq, "qp"), (pop, "op"),
                           (pav, "av"), (pav, "av")][j]
            kps.append(kpool.tile([128, S], F32, tag=ktag, name=f"kp{j}"))
        for c in range(CJ):
            for j in range(EJ):
                nc.tensor.matmul(kps[j][:],
                                 wk_sb[c][:, 128 * j:128 * (j + 1)],
                                 ytp[c], start=(c == 0), stop=(c == CJ - 1))
        for j in range(EJ):
            for fi, (h, jj, p0, p1) in enumerate(FRAGS):
                if jj != j:
                    continue
                nc.vector.tensor_scalar(kstage[fi][:], kps[j][:],
                                        bkt[:, j:j + 1], kmask[:, fi:fi + 1],
                                        mybir.AluOpType.add,
                                        mybir.AluOpType.mult)

        # ---- V projection (+bias via K=1 ones matmul) -> vb fragments ----
        vb = [cpool.tile([S, 128], BF16, tag=f"vb{fi}", name=f"vb{fi}")
              for fi in range(NF)]
        if True:  # (kept indented to match the emitted-block structure)
            for (n0, n1) in ((0, 512), (512, E)):
                vp = psc.tile([S, n1 - n0], F32, tag="sc")
                for c in range(CJ):
                    nc.tensor.matmul(vp[:], ytp[c], wv_sb[c][:, n0:n1],
                                     start=(c == 0), stop=False)
                nc.tensor.matmul(vp[:], ones77[:], bvr[:, n0:n1],
                                 start=False, stop=True)
                for fi, (h, j, p0, p1) in enumerate(FRAGS):
                    c0 = 128 * j
                    if not (n0 <= c0 and c0 + 128 <= n1):
                        continue
                    nc.vector.tensor_mul(vb[fi][:],
                                         vp[:, c0 - n0:c0 - n0 + 128],
                                         vmask[:, 128 * fi:128 * (fi + 1)])

        # ---- main loop over token chunks, software-pipelined ----
        def emit_scores(qs, tw, first_chunk=False):
            s8 = ps8.tile([H, TC], F32, tag="s8")
            aps = []
            for h in range(H):
                frs = [(fi, f) for fi, f in enumerate(FRAGS) if f[0] == h]
                if first_chunk and h < 2:
                    # psc banks are still being drained by V-proj's mask
                    # multiplies; pq/pop are free after K-proj
                    spool, stag = [(pq, "qp"), (pop, "op")][h]
                    sc = spool.tile([S, TC], F32, tag=stag, name=f"sc{h}")
                else:
                    sc = psc.tile([S, TC], F32, tag="sc")
                for i, (fi, (hh, j, p0, p1)) in enumerate(frs):
                    nc.tensor.matmul(sc[0:S, 0:tw], kstage[fi][:],
                                     qs[j][0:128, 0:tw],
                                     start=(i == 0), stop=(i == len(frs) - 1))
                a = apool.tile([S, TC], BF16, tag=f"a{h}")
                nc.scalar.activation(a[0:S, 0:tw], sc[0:S, 0:tw], AF.Exp,
                                     scale=SCALE)
                nc.tensor.matmul(s8[0:H, 0:tw], ecols[:, H * h:H * h + H],
                                 a[0:S, 0:tw],
                                 start=(h == 0), stop=(h == H - 1),
                                 skip_group_check=True)
                aps.append(a)
            return s8, aps

        def emit_recip(s8, tw):
            # approx 1/Z (fp32) + bf16 round + ONE flatten DMA of all 8
            # head rows into [1, H*tw] (partition_broadcast reads p0 only)
            rec8 = rpool.tile([H, TC], F32, tag="rec8")
            nc.vector.reciprocal_approx_fast(rec8[0:H, 0:tw], s8[0:H, 0:tw])
            rec8b = rpool.tile([H, TC], BF16, tag="rec8b")
            with nc.allow_low_precision(reason="softmax recip in bf16"):
                nc.vector.tensor_copy(rec8b[0:H, 0:tw], rec8[0:H, 0:tw])
            return rec8b

        def emit_bcast_dma(rec8b, it, tw):
            # bounce the [H, tw] reciprocal rows through DRAM, reloading
            # with a 0-stride partition dim: one store + one broadcast-load
            # replaces 8 serial Pool partition_broadcasts (Tile tracks the
            # DRAM store->load hazard through the DMA semaphores)
            zsl = dr["zs"][0:H, TC * it:TC * it + tw]
            nc.sync.dma_start(zsl, rec8b[0:H, 0:tw])
            bca = bcpool.tile([S, H, TC], BF16, tag="bca")
            nc.sync.dma_start(bca[:, :, 0:tw], zsl.partition_broadcast(S))
            return bca

        def emit_mult(aps, bca, tw):
            for h in range(H):
                nc.vector.tensor_mul(aps[h][0:S, 0:tw], aps[h][0:S, 0:tw],
                                     bca[:, h, 0:tw])

        def emit_mult_sel(aps, rec8b, tw):
            # last chunks: broadcast via PE sel-matmul (no Qproj left on PE
            # to hide the Pool-broadcast latency behind)
            for h in range(H):
                bc = psc.tile([S, TC], F32, tag="sc")
                nc.tensor.matmul(bc[0:S, 0:tw], sel8_sb[:, S * h:S * (h + 1)],
                                 rec8b[0:H, 0:tw])
                nc.vector.tensor_mul(aps[h][0:S, 0:tw], aps[h][0:S, 0:tw],
                                     bc[0:S, 0:tw])

        def emit_zrec(rec8b, tw):
            # tail-chunk path: per-feature-tile reciprocal maps
            # zrec_j = selj^T rec8b (5 PE matmuls), staged to SBUF bf16 via
            # Act copies (HW allows only ONE PSUM operand per DVE op, so the
            # later av*zrec multiply needs zrec in SBUF). Emitted before the
            # filler O-projection, so the copies are latency-hidden.
            zrs = []
            for j in range(EJ):
                zp = psc.tile([128, TC], F32, tag="sc", name=f"zrp{j}")
                nc.tensor.matmul(zp[0:128, 0:tw],
                                 selj_sb[:, 128 * j:128 * (j + 1)],
                                 rec8b[0:H, 0:tw])
                z = bcpool.tile([128, TC], BF16, tag=f"zr{j}")
                nc.scalar.activation(z[0:128, 0:tw], zp[0:128, 0:tw],
                                     AF.Copy)
                zrs.append(z)
            return zrs

        def emit_av(aps, tw, zrs=None):
            # zrs given: aps are UNnormalized; the softmax division folds
            # into the PSUM->SBUF multiply (replacing the Act copy)
            attn = []
            for j in range(EJ):
                if zrs is not None and j == 2:
                    # third bank (pop, idle during tail AV) breaks the
                    # copy->mult->recycle WAR chain on the pav pair
                    av = pop.tile([128, TC], F32, tag="op", name="av2")
                else:
                    av = pav.tile([128, TC], F32, tag="av")
                frs = [(fi, f) for fi, f in enumerate(FRAGS) if f[1] == j]
                for i, (fi, (h, jj, p0, p1)) in enumerate(frs):
                    nc.tensor.matmul(av[0:128, 0:tw], vb[fi][:],
                                     aps[h][0:S, 0:tw],
                                     start=(i == 0), stop=(i == len(frs) - 1))
                at = atpool.tile([128, TC], BF16, tag=f"at{j}")
                if zrs is not None:
                    with nc.allow_low_precision(reason="attn tile in bf16"):
                        nc.vector.tensor_mul(at[0:128, 0:tw],
                                             av[0:128, 0:tw],
                                             zrs[j][0:128, 0:tw])
                else:
                    nc.scalar.activation(at[0:128, 0:tw], av[0:128, 0:tw],
                                         AF.Copy)
                attn.append(at)
            return attn

        def emit_oproj(attn, it, t0, tw):
            # O projection + bias; batched out DMA except on the last chunk
            # (per-tile DMAs there so the store drain starts earlier).
            last = it == NT - 1
            ob5 = None if last else opool.tile([128, EJ, TC], BF16, tag="ob")
            for p in range(EJ):
                if it >= NT - 3 and p % 2 == 0:
                    op = pq.tile([128, TC], F32, tag="qp", name=f"opq{p}")
                elif it < NT - 3 and p % 2 == 0:
                    op = pav.tile([128, TC], F32, tag="av", name=f"opv{p}")
                else:
                    op = pop.tile([128, TC], F32, tag="op")
                for j in range(EJ):
                    nc.tensor.matmul(op[0:128, 0:tw],
                                     wo_sb[j][:, 128 * p:128 * (p + 1)],
                                     attn[j][0:128, 0:tw],
                                     start=(j == 0), stop=(j == EJ - 1))
                if last:
                    ob = opool.tile([128, TC], BF16, tag=f"obl{p}")
                    nc.scalar.activation(ob[0:128, 0:tw], op[0:128, 0:tw],
                                         AF.Identity, bias=bot[:, p:p + 1])
                    nc.sync.dma_start(
                        dr["ot"][128 * p:128 * (p + 1), t0:t0 + tw],
                        ob[0:128, 0:tw])
                else:
                    nc.scalar.activation(ob5[:, p, 0:tw], op[0:128, 0:tw],
                                         AF.Identity, bias=bot[:, p:p + 1])
            # the store DMA is flushed by the caller AFTER the next chunk's
            # broadcast-load is issued, so it queues behind it on the wire
            return None if last else (ob5, t0, tw)

        def flush_out(pending):
            if pending is not None:
                ob5, t0, tw = pending
                nc.scalar.dma_start(
                    dr["ot"][:, t0:t0 + tw].rearrange("(b p) c -> p b c",
                                                      p=128),
                    ob5[:, :, 0:tw])

        # Uniform pipeline: every chunk's softmax-normalization latency is
        # hidden behind ~5us of independent PE work — Qproj_{i+2} while it
        # exists, then the previous chunk's deferred O-projection.
        deferred = None          # (attn, it, t0, tw) awaiting O-projection
        pending_out = None       # (ob5, t0, tw) awaiting its store DMA
        for it in range(NT):
            t0, tw = CHUNKS[it]
            if it + 2 < NT:
                x_fut = load_x(it + 2)
            s8, aps = emit_scores(qs_cur, tw, first_chunk=(it == 0))
            rec8b = emit_recip(s8, tw)
            use_post = it >= NT - 2
            bca = None if use_post else emit_bcast_dma(rec8b, it, tw)
            zrs = emit_zrec(rec8b, tw) if use_post else None
            flush_out(pending_out)
            pending_out = None
            if it + 2 < NT:
                qs_cur, qs_nxt = qs_nxt, emit_qproj(x_fut, CHUNKS[it + 2][1])
            else:
                qs_cur = qs_nxt
                if deferred is not None:
                    pending_out = emit_oproj(*deferred)
                    deferred = None
            if not use_post:
                emit_mult(aps, bca, tw)
            if it == NT - 1 and pending_out is not None:
                flush_out(pending_out)
                pending_out = None
            attn = emit_av(aps, tw, zrs=zrs)
            if it < NT - 3:
                pending_out = emit_oproj(attn, it, t0, tw)
            elif it == NT - 1:
                emit_oproj(attn, it, t0, tw)
            else:
                deferred = (attn, it, t0, tw)
        flush_out(pending_out)


def build_program(iters=1):
    nc = bacc.Bacc("TRN2", target_bir_lowering=False, debug=False, num_devices=B)
    dr = {}

    def din(name, shape, dt=BF16):
        dr[name] = nc.dram_tensor(name, shape, dt, kind="ExternalInput")
        return dr[name]

    din("xt", [E, T])
    din("yt", [CR, S])
    din("wq", [E, E])
    din("wk", [CR, E])
    din("wv", [CR, E])
    din("wo", [E, E])
    din("fc", [128, 15 + NF], F32)
    din("fb1", [1, E + S])
    din("e77", [S, H * H + H * S + 128 * NF + E])
    dr["zs"] = nc.dram_tensor("zs", [H, NT * TC], BF16, kind="Internal")
    dr["ot"] = nc.dram_tensor("ot", [E, T], BF16, kind="ExternalOutput")

    with tile.TileContext(nc) as tc:
        for _ in range(iters):
            _emit(nc, tc, {k: v[:] for k, v in dr.items()})
    nc.compile()
    return nc


def make_in_maps(x, y, Wq, bq, Wk, bk, Wv, bv, Wo, bo):
    bf = mybir.dt.np(BF16)
    f32 = np.float32
    fb = lambda a: np.ascontiguousarray(np.asarray(a, dtype=f32).astype(bf))
    ecols = np.zeros((S, H * H), f32)
    for h in range(H):
        ecols[:, H * h + h] = 1.0
    sel8 = np.zeros((H, S * H), f32)
    for h in range(H):
        sel8[h, S * h:S * (h + 1)] = 1.0
    # sel8 lives in the [S,*] fused constant tensor: only rows 0..H-1 used
    sel8_pad = np.zeros((S, H * S), f32)
    sel8_pad[0:H, :] = sel8
    kmask = np.zeros((128, NF), f32)
    vmask = np.zeros((S, 128 * NF), f32)
    for fi, (h, j, p0, p1) in enumerate(FRAGS):
        kmask[p0:p1, fi] = 1.0
        vmask[:, 128 * fi + p0:128 * fi + p1] = 1.0
    fc = np.concatenate([
        np.asarray(bq, f32).reshape(EJ, 128).T,
        np.asarray(bk, f32).reshape(EJ, 128).T,
        np.asarray(bo, f32).reshape(EJ, 128).T,
        kmask,
    ], axis=1)
    fb1 = np.concatenate([np.asarray(bv, f32).reshape(1, E),
                          np.ones((1, S), f32)], axis=1)
    selj = np.zeros((S, E), f32)
    for e in range(E):
        selj[e // D, e] = 1.0          # rows 0..H-1 used (head of feature e)
    e77 = np.concatenate([ecols, sel8_pad, vmask, selj], axis=1)
    shared = dict(
        wq=fb(Wq), wk=fb(Wk), wv=fb(Wv), wo=fb(Wo),
        fc=np.ascontiguousarray(fc),
        fb1=fb(fb1),
        e77=fb(e77),
    )
    x = np.asarray(x, f32)
    y = np.asarray(y, f32)
    in_maps = []
    for b in range(B):
        m = dict(shared)
        m["xt"] = fb(x[b].T)
        m["yt"] = fb(y[b].T)
        in_maps.append(m)
    return in_maps


def assemble_output(results):
    return np.stack(
        [np.asarray(results[b]["ot"], dtype=np.float32).T for b in range(B)],
        axis=0)


_PROG = None


def _prog():
    global _PROG
    if _PROG is None:
        _PROG = build_program()
    return _PROG


def kernel(x, y, Wq, bq, Wk, bk, Wv, bv, Wo, bo):
    nc = _prog()
    in_maps = make_in_maps(x, y, Wq, bq, Wk, bk, Wv, bv, Wo, bo)
    res = run_bass_kernel_spmd(nc, in_maps, core_ids=list(range(B)))
    return assemble_output(res.results)
